# revision 4
# baseline (speedup 1.0000x reference)
"""Trainium2 Bass kernel for im2col conv2d + bias + channel-pack.

Semantics (matches the reference):
    out[c, w] = sum_k enc_x[w, k] * weight[c, k] + bias[c],  flattened to [C*W].

Strategy (v3):
  - Shard the window dimension W=1048576 across 8 cores (131072 windows each).
  - fp16 I/O halves HBM traffic (PE accumulates fp32); rel err ~3e-4.
  - Stationary operand is a block-diagonal [98, 128] weight matrix: rows
    0..48 = chunk-A k-values, 49..97 = chunk-B, so one moving column covers
    TWO windows; two column-group matmuls (tile_position cols 0/64) run
    concurrently, each N=512 into its own half of a [128, 1024] psum tile.
  - Fine-grained psum pipeline: [128, 1024] fp32 tiles are 2 PSUM banks,
    bufs=4 covers all 8 banks; the bias+copy alternates between the scalar
    ACT engine (activation Identity + bias) and the vector DVE engine
    (tensor_scalar_add) so no single ~1us copy serializes the PE.
  - Stores are ONE [128, f/2] DMA per iteration (partition = group*32+chan)
    hitting both SBUF AXI port halves; input rides the scalar HWDGE ring
    (rows 0..47) + gpsimd SWDGE (rows 48..97), stores ride the sync ring,
    so the 16 SDMA engines round-robin three streams continuously.
"""

import os

import numpy as np

K = 49
C = 32
WINDOWS_NB = 1048576
N_CORES = 8
W_CORE = WINDOWS_NB // N_CORES  # 131072

F = int(os.environ.get("BASS_KERNEL_F", "4096"))  # x-columns per tile

_PROGRAM_CACHE: dict = {}
LAST_RESULT = None  # BassKernelResults of the most recent run (for test harness)


def build_program(w_core=W_CORE, f=F):
    import concourse.tile as tile
    from concourse import bacc, mybir

    assert w_core % (2 * f) == 0 and f % 2048 == 0
    n_outer = w_core // (2 * f)
    nq = f // 2048  # psum tiles per outer iteration

    nc = bacc.Bacc("TRN2", debug=False, num_devices=N_CORES)
    # Host-shuffled fp16 input shards (see prepare_inputs for the layout).
    xt = nc.dram_tensor("xt", [n_outer, 2 * K, f], mybir.dt.float16, kind="ExternalInput")
    w4 = nc.dram_tensor("w4", [2 * K, 4 * C], mybir.dt.float16, kind="ExternalInput")
    br = nc.dram_tensor("br", [4 * C, 1], mybir.dt.float32, kind="ExternalInput")
    # fp16 output (upcast on host).
    out = nc.dram_tensor("out", [C, w_core], mybir.dt.float16, kind="ExternalOutput")

    with tile.TileContext(nc) as tc:
        with tc.tile_pool(name="const", bufs=1) as cpool, \
             tc.tile_pool(name="xin", bufs=4) as xpool, \
             tc.tile_pool(name="osb", bufs=3) as opool, \
             tc.tile_pool(name="ps", bufs=4, space="PSUM") as ppool:
            w_sb = cpool.tile([2 * K, 4 * C], mybir.dt.float16)
            nc.sync.dma_start(out=w_sb, in_=w4.ap())
            b_sb = cpool.tile([4 * C, 1], mybir.dt.float32)
            nc.sync.dma_start(out=b_sb, in_=br.ap())

            xt_ap = xt.ap()
            # out element [c, w]; w = g*(w_core/4) + it*(f/2) + u where the
            # o_tile partition is g*32+c and u is the o_tile column: every
            # store is one contiguous-per-partition [128, f/2] AP spanning
            # both SBUF AXI port halves.
            out_r = out.ap().rearrange(
                "c (g i u) -> i g c u", g=4, i=n_outer, u=f // 2,
            )

            cp = 0  # psum tile counter (for ACT/DVE alternation)
            for it in range(n_outer):
                x_tile = xpool.tile([2 * K, f], mybir.dt.float16)
                if it == 0:
                    # Fast ramp: spread the first tile over both HWDGE rings
                    # (SWDGE Q7 is slow to start).
                    nc.sync.dma_start(out=x_tile[0:48, :], in_=xt_ap[it, 0:48])
                    nc.scalar.dma_start(out=x_tile[48:96, :], in_=xt_ap[it, 48:96])
                    nc.gpsimd.dma_start(out=x_tile[96:2 * K, :], in_=xt_ap[it, 96:2 * K])
                else:
                    # 48 rows fan HWDGE descriptors over all 16 engines;
                    # SWDGE sprays the remaining 50 by partition port.
                    nc.scalar.dma_start(out=x_tile[0:48, :], in_=xt_ap[it, 0:48])
                    nc.gpsimd.dma_start(out=x_tile[48:2 * K, :], in_=xt_ap[it, 48:2 * K])
                o_tile = opool.tile([4 * C, f // 2], mybir.dt.float16)
                for q in range(nq):
                    ps = ppool.tile([4 * C, 1024], mybir.dt.float32)
                    c0 = q * 2048
                    for vb in range(2):
                        pc = slice(vb * 512, (vb + 1) * 512)
                        xb = c0 + vb * 1024
                        # concurrent MM pair on PE column groups 0-1 / 2-3
                        nc.tensor.matmul(
                            ps[0:2 * C, pc], w_sb[:, 0:2 * C],
                            x_tile[:, xb:xb + 512],
                            start=True, stop=True,
                            tile_position=(0, 0),
                        )
                        nc.tensor.matmul(
                            ps[2 * C:4 * C, pc], w_sb[:, 2 * C:4 * C],
                            x_tile[:, xb + 512:xb + 1024],
                            start=True, stop=True,
                            tile_position=(0, 2 * C),
                        )
                    o_sl = o_tile[:, q * 1024:(q + 1) * 1024]
                    if cp % 2 == 0:
                        nc.scalar.activation(
                            o_sl, ps, mybir.ActivationFunctionType.Identity,
                            bias=b_sb, scale=1.0,
                        )
                    else:
                        nc.vector.tensor_scalar_add(o_sl, ps, b_sb)
                    cp += 1
                nc.sync.dma_start(out=out_r[it], in_=o_tile)
    nc.compile()
    return nc


def _get_program():
    key = (W_CORE, F)
    if key not in _PROGRAM_CACHE:
        _PROGRAM_CACHE[key] = build_program()
    return _PROGRAM_CACHE[key]


def prepare_inputs(enc_x, weight, bias, f=F):
    """Host-side prep: per-core shuffled fp16 shards + block-diag weights.

    Window mapping (per core): canonical window index
        w = gh*65536 + ch*32768 + it*(f/2) + q*1024 + vb*512 + t
    lands at x-tile column  X = q*2048 + vb*1024 + gh*512 + t  of iteration
    it, in x-tile row ch*49 + k, and at o_tile partition (2*gh+ch)*32 + c.
    """
    enc_x = np.asarray(enc_x, dtype=np.float32)
    weight = np.asarray(weight, dtype=np.float32)
    bias = np.asarray(bias, dtype=np.float32)
    n_outer = W_CORE // (2 * f)

    wT = weight.reshape(C, K).T.astype(np.float16)  # [49, 32]
    w4 = np.zeros((2 * K, 4 * C), dtype=np.float16)
    for cg in range(2):
        for ch in range(2):
            w4[ch * K:(ch + 1) * K, cg * 64 + ch * 32:cg * 64 + ch * 32 + 32] = wT
    brr = np.tile(bias, 4)[:, None].astype(np.float32)

    x16 = enc_x.astype(np.float16)
    shards = []
    for i in range(N_CORES):
        sh = np.ascontiguousarray(x16[i * W_CORE:(i + 1) * W_CORE].T)  # [49, 131072]
        # w axis -> (gh, ch, it, q, vb, t)
        arr = sh.reshape(K, 2, 2, n_outer, f // 2048, 2, 512)
        perm = arr.transpose(3, 2, 0, 4, 5, 1, 6)  # (it, ch, k, q, vb, gh, t)
        shards.append(np.ascontiguousarray(perm).reshape(n_outer, 2 * K, f))
    return shards, w4, brr


def kernel(enc_x, weight, bias, windows_nb=None):
    global LAST_RESULT
    from concourse import bass_utils

    shards, w4, brr = prepare_inputs(enc_x, weight, bias)
    nc = _get_program()
    in_maps = [{"xt": shards[i], "w4": w4, "br": brr} for i in range(N_CORES)]
    trace = bool(int(os.environ.get("BASS_KERNEL_TRACE", "0")))
    tmpdir = os.environ.get("BASS_KERNEL_TMPDIR") or None
    res = bass_utils.run_bass_kernel_spmd(
        nc, in_maps, core_ids=list(range(N_CORES)), trace=trace, tmpdir=tmpdir
    )
    LAST_RESULT = res
    outs = [res.results[i]["out"] for i in range(N_CORES)]
    return np.concatenate(outs, axis=1).astype(np.float32).reshape(-1)


# revision 7
# speedup vs baseline: 1.3959x; 1.3959x over previous
"""Trainium2 Bass kernel for im2col conv2d + bias + channel-pack.

Semantics (matches the reference):
    out[c, w] = sum_k enc_x[w, k] * weight[c, k] + bias[c],  flattened to [C*W].

Strategy (v3):
  - Shard the window dimension W=1048576 across 8 cores (131072 windows each).
  - fp16 I/O halves HBM traffic (PE accumulates fp32); rel err ~3e-4.
  - Stationary operand is a block-diagonal [98, 128] weight matrix: rows
    0..48 = chunk-A k-values, 49..97 = chunk-B, so one moving column covers
    TWO windows; two column-group matmuls (tile_position cols 0/64) run
    concurrently, each N=512 into its own half of a [128, 1024] psum tile.
  - Fine-grained psum pipeline: [128, 1024] fp32 tiles are 2 PSUM banks,
    bufs=4 covers all 8 banks; the bias+copy alternates between the scalar
    ACT engine (activation Identity + bias) and the vector DVE engine
    (tensor_scalar_add) so no single ~1us copy serializes the PE.
  - Stores are ONE [128, f/2] DMA per iteration (partition = group*32+chan)
    hitting both SBUF AXI port halves; input rides the scalar HWDGE ring
    (rows 0..47) + gpsimd SWDGE (rows 48..97), stores ride the sync ring,
    so the 16 SDMA engines round-robin three streams continuously.
"""

import os

import numpy as np

K = 49
C = 32
WINDOWS_NB = 1048576
N_CORES = 8
W_CORE = WINDOWS_NB // N_CORES  # 131072

F = int(os.environ.get("BASS_KERNEL_F", "8192"))  # x-columns per tile

_PROGRAM_CACHE: dict = {}
LAST_RESULT = None  # BassKernelResults of the most recent run (for test harness)


def build_program(w_core=W_CORE, f=F):
    import concourse.tile as tile
    from concourse import bacc, mybir

    assert w_core % (2 * f) == 0 and f % 2048 == 0
    n_outer = w_core // (2 * f)
    nq = f // 2048  # psum tiles per outer iteration

    nc = bacc.Bacc("TRN2", debug=False, num_devices=N_CORES)
    # Host-shuffled fp16 input shards (see prepare_inputs for the layout).
    xt = nc.dram_tensor("xt", [n_outer, 2 * K, f], mybir.dt.float16, kind="ExternalInput")
    w4 = nc.dram_tensor("w4", [2 * K, 4 * C], mybir.dt.float16, kind="ExternalInput")
    br = nc.dram_tensor("br", [4 * C, 1], mybir.dt.float32, kind="ExternalInput")
    # fp16 output (upcast on host).
    out = nc.dram_tensor("out", [C, w_core], mybir.dt.float16, kind="ExternalOutput")

    with tile.TileContext(nc) as tc:
        with tc.tile_pool(name="const", bufs=1) as cpool, \
             tc.tile_pool(name="xin", bufs=4) as xpool, \
             tc.tile_pool(name="osb", bufs=3) as opool, \
             tc.tile_pool(name="ps", bufs=4, space="PSUM") as ppool:
            w_sb = cpool.tile([2 * K, 4 * C], mybir.dt.float16)
            nc.sync.dma_start(out=w_sb, in_=w4.ap())
            b_sb = cpool.tile([4 * C, 1], mybir.dt.float32)
            nc.sync.dma_start(out=b_sb, in_=br.ap())

            xt_ap = xt.ap()
            # out element [c, w]; w = g*(w_core/4) + (it//2)*f + u where the
            # o_tile partition is g*32+c and u is the o_tile column (o_tile
            # spans TWO iterations).  Each store is one [c=32, u] AP: the
            # 32-row outer dim fans HWDGE descriptors over all 16 engines,
    # and the 4 back-to-back stores (g=0..3) cover both port halves.
            assert n_outer % 2 == 0
            out_r = out.ap().rearrange(
                "c (g i u) -> i g c u", g=4, i=n_outer // 2, u=f,
            )

            cp = 0  # psum tile counter (for ACT/DVE alternation)
            o_tile = None
            for it in range(n_outer):
                # Every input DMA spans both SBUF AXI port halves (partitions
                # 0-63 -> even ports, 64-127 -> odd): scalar takes 32 even +
                # 16 odd rows, gpsimd the rest.  Row counts on the HWDGE ring
                # are multiples of 16 so descriptors fan over all 16 engines.
                x_tile = xpool.tile([2 * K, f], mybir.dt.float16)
                nc.scalar.dma_start(out=x_tile[0:32, :], in_=xt_ap[it, 0:32])
                nc.scalar.dma_start(out=x_tile[64:80, :], in_=xt_ap[it, 64:80])
                if it == 0:
                    # Fast ramp: spread the first tile over both HWDGE rings
                    # (SWDGE Q7 is slow to start).
                    nc.sync.dma_start(out=x_tile[32:64, :], in_=xt_ap[it, 32:64])
                    nc.gpsimd.dma_start(out=x_tile[80:2 * K, :], in_=xt_ap[it, 80:2 * K])
                else:
                    nc.gpsimd.dma_start(out=x_tile[32:64, :], in_=xt_ap[it, 32:64])
                    nc.gpsimd.dma_start(out=x_tile[80:2 * K, :], in_=xt_ap[it, 80:2 * K])
                if it % 2 == 0:
                    o_tile = opool.tile([4 * C, f], mybir.dt.float16)
                for q in range(nq):
                    ps = ppool.tile([4 * C, 1024], mybir.dt.float32)
                    c0 = q * 2048
                    for vb in range(2):
                        pc = slice(vb * 512, (vb + 1) * 512)
                        xb = c0 + vb * 1024
                        # concurrent MM pair on PE column groups 0-1 / 2-3
                        nc.tensor.matmul(
                            ps[0:2 * C, pc], w_sb[:, 0:2 * C],
                            x_tile[:, xb:xb + 512],
                            start=True, stop=True,
                            tile_position=(0, 0),
                        )
                        nc.tensor.matmul(
                            ps[2 * C:4 * C, pc], w_sb[:, 2 * C:4 * C],
                            x_tile[:, xb + 512:xb + 1024],
                            start=True, stop=True,
                            tile_position=(0, 2 * C),
                        )
                    ob = (it % 2) * (f // 2) + q * 1024
                    o_sl = o_tile[:, ob:ob + 1024]
                    if cp % 2 == 0:
                        nc.scalar.activation(
                            o_sl, ps, mybir.ActivationFunctionType.Identity,
                            bias=b_sb, scale=1.0,
                        )
                    else:
                        nc.vector.tensor_scalar_add(o_sl, ps, b_sb)
                    cp += 1
                if it % 2 == 1:
                    for g in range(4):
                        nc.sync.dma_start(
                            out=out_r[it // 2, g],
                            in_=o_tile[g * C:(g + 1) * C, :],
                        )
    nc.compile()
    return nc


def _get_program():
    key = (W_CORE, F)
    if key not in _PROGRAM_CACHE:
        _PROGRAM_CACHE[key] = build_program()
    return _PROGRAM_CACHE[key]


def prepare_inputs(enc_x, weight, bias, f=F):
    """Host-side prep: per-core shuffled fp16 shards + block-diag weights.

    Window mapping (per core): canonical window index
        w = gh*65536 + ch*32768 + it*(f/2) + q*1024 + vb*512 + t
    lands at x-tile column  X = q*2048 + vb*1024 + gh*512 + t  of iteration
    it, in x-tile row ch*49 + k, and at o_tile partition (2*gh+ch)*32 + c.
    """
    enc_x = np.asarray(enc_x, dtype=np.float32)
    weight = np.asarray(weight, dtype=np.float32)
    bias = np.asarray(bias, dtype=np.float32)
    n_outer = W_CORE // (2 * f)

    wT = weight.reshape(C, K).T.astype(np.float16)  # [49, 32]
    w4 = np.zeros((2 * K, 4 * C), dtype=np.float16)
    for cg in range(2):
        for ch in range(2):
            w4[ch * K:(ch + 1) * K, cg * 64 + ch * 32:cg * 64 + ch * 32 + 32] = wT
    brr = np.tile(bias, 4)[:, None].astype(np.float32)

    x16 = enc_x.astype(np.float16)
    shards = []
    for i in range(N_CORES):
        sh = np.ascontiguousarray(x16[i * W_CORE:(i + 1) * W_CORE].T)  # [49, 131072]
        # w axis -> (gh, ch, it, q, vb, t)
        arr = sh.reshape(K, 2, 2, n_outer, f // 2048, 2, 512)
        perm = arr.transpose(3, 2, 0, 4, 5, 1, 6)  # (it, ch, k, q, vb, gh, t)
        shards.append(np.ascontiguousarray(perm).reshape(n_outer, 2 * K, f))
    return shards, w4, brr


def kernel(enc_x, weight, bias, windows_nb=None):
    global LAST_RESULT
    from concourse import bass_utils

    shards, w4, brr = prepare_inputs(enc_x, weight, bias)
    nc = _get_program()
    in_maps = [{"xt": shards[i], "w4": w4, "br": brr} for i in range(N_CORES)]
    trace = bool(int(os.environ.get("BASS_KERNEL_TRACE", "0")))
    tmpdir = os.environ.get("BASS_KERNEL_TMPDIR") or None
    res = bass_utils.run_bass_kernel_spmd(
        nc, in_maps, core_ids=list(range(N_CORES)), trace=trace, tmpdir=tmpdir
    )
    LAST_RESULT = res
    outs = [res.results[i]["out"] for i in range(N_CORES)]
    return np.concatenate(outs, axis=1).astype(np.float32).reshape(-1)
